# revision 1
# baseline (speedup 1.0000x reference)
"""Styled 3D conv (StyleGAN-style modulated conv3d) on 8 Trainium2 NeuronCores.

Reference computation:
  st = s @ style_weight.T + style_bias                 (N, Cin)
  w  = weight[None] * st[:, None, :, None*3]           (N, Cout, Cin, 3,3,3)
  w  = w * rsqrt(sum(w^2 over (Cin,kd,kh,kw)) + eps)   per-sample demodulated
  y  = grouped_conv3d(x, w, groups=N, VALID) + bias    (N, Cout, 62,62,62)

Shapes: x (4, 64, 64, 64, 64) f32, y (4, 64, 62, 62, 62) f32.

Strategy (8 cores = 4 samples x 2 depth-halves):
  - Host: compute the tiny per-sample modulated weights (fp32), pack into
    matmul lhsT blocks, cast x + weights to bf16.
  - Each core gets 34 input depth-planes and produces 31.5 output planes:
    the odd core of each sample receives its shard depth+height MIRRORED
    (with kd/kh taps mirrored in its lhsT), so one uniform SPMD program
    splits the 62 output planes 31/31 with no redundant compute; the
    half-covered boundary pair is merged on gather.
  - SBUF x layout: "slot" q = [128 partitions, 64, 64] bf16 with
    partitions 0:64  = (ci, even plane 2q), 64:128 = (ci, odd plane 2q+1).
  - Output plane pair (d, d+1) = (2j, 2j+1) is computed with M=128 packing
    (psum partitions 0:64 -> y[d] Cout, 64:128 -> y[d+1] Cout) by 18 matmul
    "streams": 2 slots (j, j+1) x 9 (kh, kw) taps, K=128 = (2 planes x ci).
    The 3-wide kd-tap band over 2x2 (plane x outplane) tiles gives 6/8
    active 64x64 quadrants -> 75% PE utilization (provably optimal for this
    shape on a 128x128 PE), with no x duplication in SBUF.
  - Per pair, the 62x62 spatial output is split into 8 PSUM banks
    (7 chunks of 8 rows + 1 of 6); 18 matmuls accumulate per bank.
    First/last pairs run chunk-major to shrink the kernel head/tail.
  - Drain: DVE tensor_scalar_add(psum, bias) -> SBUF staging -> DMA to DRAM.

Measured: 483 us HW exec per core (PE matmul stream at its 207 ns/496-col
floor), rel l2 error vs fp32 reference 2.2e-03.
"""

import numpy as np
import ml_dtypes

import concourse.mybir as mybir
import concourse.tile as tile
from concourse import bacc
from concourse.bass_utils import run_bass_kernel_spmd

EPS = 1e-8
N_CORES = 8
N, CIN, COUT, D = 4, 64, 64, 64
DO = D - 2              # 62 output planes/rows/cols
PLANES_IN = 34          # input planes per core
PAIRS = 16              # output plane pairs per core (32 planes)
SLOTS = PLANES_IN // 2  # 17
ROWS_PER_CHUNK = 8
CHUNKS = 8              # 7*8 + 6 = 62 rows
BF16 = mybir.dt.bfloat16
F32 = mybir.dt.float32

_compiled = {}


def _build_nc():
    nc = bacc.Bacc("TRN2", target_bir_lowering=False, debug=False,
                   num_devices=N_CORES)
    xs = nc.dram_tensor("xs", [CIN, PLANES_IN, D, D], BF16,
                        kind="ExternalInput").ap()
    wts = nc.dram_tensor("wts", [128, 18 * 128], BF16,
                         kind="ExternalInput").ap()
    b128 = nc.dram_tensor("b128", [128, 1], F32, kind="ExternalInput").ap()
    y = nc.dram_tensor("y", [COUT, 2 * PAIRS, DO * DO], F32,
                       kind="ExternalOutput").ap()

    with tile.TileContext(nc) as tc:
        with (
            tc.tile_pool(name="wp", bufs=1) as wpool,
            tc.tile_pool(name="xp", bufs=SLOTS) as xpool,
            tc.tile_pool(name="ps", bufs=CHUNKS, space="PSUM") as pspool,
            tc.tile_pool(name="st", bufs=2) as stpool,
        ):
            w_sb = wpool.tile([128, 18 * 128], BF16)
            # split the weight load across queues — it gates the first matmul
            for i in range(4):
                o = i * 576
                nc.sync.dma_start(w_sb[:, o:o + 576], wts[:, o:o + 576])
            bias_sb = wpool.tile([128, 1], F32)
            nc.sync.dma_start(bias_sb[:, :], b128[:, :])

            slots = []
            for q in range(SLOTS):
                t = xpool.tile([128, D, D], BF16, tag="xslot",
                               name=f"xslot_{q}")
                if q < 2:
                    # row-split the first two slots so the head of the kernel
                    # (pair 0, chunk-major) can start after ~10 rows arrive
                    for r in range(0, D, 16):
                        nc.sync.dma_start(t[0:64, r:r + 16],
                                          xs[:, 2 * q, r:r + 16])
                        nc.sync.dma_start(t[64:128, r:r + 16],
                                          xs[:, 2 * q + 1, r:r + 16])
                else:
                    nc.sync.dma_start(t[0:64], xs[:, 2 * q])
                    nc.sync.dma_start(t[64:128], xs[:, 2 * q + 1])
                slots.append(t)

            # PE warmup: the HAM clock gate holds the PE at 1.2 GHz until
            # ~3.4us of sustained activity. Run garbage matmuls on a scratch
            # PSUM bank while the first slots' DMAs are still in flight, so
            # the real matmul stream starts at 2.4 GHz. The scratch bank is
            # reclaimed by pair 0 (start=True clears it); results never read.
            warm_src = wpool.tile([128, 496], BF16, name="warm_src")
            nc.vector.memset(warm_src[:, :], 0.0)
            warm_ps = pspool.tile([128, 496], F32, tag="ps", name="warm_ps")
            for _ in range(250):
                nc.tensor.matmul(warm_ps[:, :64], warm_src[:, 0:128],
                                 warm_src[:, 128:192], start=True, stop=True)

            def stream_iter(j):
                for sl, var in ((j, 0), (j + 1, 1)):
                    for kh in range(3):
                        for kw in range(3):
                            blk = (kh * 3 + kw) * 2 + var
                            yield w_sb[:, blk * 128:(blk + 1) * 128], sl, kh, kw

            def rhs_ap(sl, kh, kw, c, rows):
                return slots[sl][:, c * ROWS_PER_CHUNK + kh:
                                 c * ROWS_PER_CHUNK + kh + rows,
                                 kw:kw + DO]

            for j in range(PAIRS):
                last = j == PAIRS - 1
                # last pair computes only rows 0..31 (chunks 0..3); the
                # mirrored partner core supplies the remaining rows.
                nchunks = 4 if last else CHUNKS
                staging = stpool.tile([128, DO * DO], F32, tag="stage")
                psums = [pspool.tile([128, 496], F32, tag="ps",
                                     name=f"ps_{j}_{c}")
                         for c in range(nchunks)]

                def drain(c):
                    rows = min(ROWS_PER_CHUNK, DO - c * ROWS_PER_CHUNK)
                    n = rows * DO
                    o = c * ROWS_PER_CHUNK * DO
                    nc.vector.tensor_scalar_add(staging[:, o:o + n],
                                                psums[c][:, :n], bias_sb)
                    return o, n

                if 0 < j < PAIRS - 1:
                    # stream-major: each lhsT loaded once, 8 matmuls per load
                    for s_idx, (lhsT, sl, kh, kw) in enumerate(stream_iter(j)):
                        for c in range(nchunks):
                            rows = min(ROWS_PER_CHUNK, DO - c * ROWS_PER_CHUNK)
                            nc.tensor.matmul(psums[c][:, :rows * DO], lhsT,
                                             rhs_ap(sl, kh, kw, c, rows),
                                             start=(s_idx == 0),
                                             stop=(s_idx == 17))
                    for c in range(nchunks):
                        drain(c)
                    nc.sync.dma_start(y[:, 2 * j], staging[0:64])
                    nc.sync.dma_start(y[:, 2 * j + 1], staging[64:128])
                else:
                    # chunk-major on the first pair (start computing before the
                    # full slots arrive) and last pair (drain + store each
                    # chunk as soon as it completes to shrink the kernel tail)
                    for c in range(nchunks):
                        rows = min(ROWS_PER_CHUNK, DO - c * ROWS_PER_CHUNK)
                        for s_idx, (lhsT, sl, kh, kw) in enumerate(stream_iter(j)):
                            nc.tensor.matmul(psums[c][:, :rows * DO], lhsT,
                                             rhs_ap(sl, kh, kw, c, rows),
                                             start=(s_idx == 0),
                                             stop=(s_idx == 17))
                        o, n = drain(c)
                        nc.sync.dma_start(y[:, 2 * j, o:o + n],
                                          staging[0:64, o:o + n])
                        nc.sync.dma_start(y[:, 2 * j + 1, o:o + n],
                                          staging[64:128, o:o + n])
    nc.compile()
    return nc


def _modulated_weights(s_n, style_weight, style_bias, weight):
    st = s_n.astype(np.float32) @ style_weight.T.astype(np.float32) + style_bias
    w = weight * st[None, :, None, None, None]
    demod = 1.0 / np.sqrt(np.sum(w * w, axis=(1, 2, 3, 4)) + EPS)
    return w * demod[:, None, None, None, None]


def _build_lhsT(wmod):
    """(18, 128, 128) fp32: block (kh*3+kw)*2+var; lhsT[k=(half,ci), m=(colhalf,co)]."""
    out = np.zeros((9, 2, 128, 128), np.float32)
    for kh in range(3):
        for kw in range(3):
            b = kh * 3 + kw
            wt = wmod[:, :, :, kh, kw]         # (co, ci, kd)
            A = out[b, 0]
            B = out[b, 1]
            A[0:64, 0:64] = wt[:, :, 0].T      # lower -> y[d],   kd0
            A[64:128, 0:64] = wt[:, :, 1].T    # upper -> y[d],   kd1
            A[64:128, 64:128] = wt[:, :, 0].T  # upper -> y[d+1], kd0
            B[0:64, 0:64] = wt[:, :, 2].T      # lower -> y[d],   kd2
            B[0:64, 64:128] = wt[:, :, 1].T    # lower -> y[d+1], kd1
            B[64:128, 64:128] = wt[:, :, 2].T  # upper -> y[d+1], kd2
    return out.reshape(18, 128, 128)


def _prepare_in_maps(x, s, style_weight, style_bias, weight, bias):
    bias128 = np.concatenate([bias.reshape(COUT), bias.reshape(COUT)])
    bias128 = np.ascontiguousarray(bias128.reshape(128, 1), np.float32)

    x_bf = x.astype(ml_dtypes.bfloat16)
    in_maps = []
    for core in range(N_CORES):
        n, half = divmod(core, 2)
        wmod = _modulated_weights(s[n], style_weight, style_bias, weight)
        if half == 0:
            xs = x_bf[n][:, 0:PLANES_IN]
        else:
            # mirrored shard: flip depth + height; kernel taps flip too,
            # so the same program computes the flipped top half
            xs = x_bf[n][:, D - PLANES_IN:D][:, ::-1, ::-1, :]
            wmod = wmod[:, :, ::-1, ::-1, :]
        lhsT = _build_lhsT(np.ascontiguousarray(wmod))  # (18, 128, 128)
        wts = np.ascontiguousarray(
            lhsT.transpose(1, 0, 2).reshape(128, 18 * 128)
        ).astype(ml_dtypes.bfloat16)
        in_maps.append({"xs": np.ascontiguousarray(xs), "wts": wts,
                        "b128": bias128})
    return in_maps


def kernel(x, s, style_weight, style_bias, weight, bias):
    x = np.asarray(x)
    s = np.asarray(s)
    style_weight = np.asarray(style_weight, np.float32)
    style_bias = np.asarray(style_bias, np.float32)
    weight = np.asarray(weight, np.float32)
    bias = np.asarray(bias, np.float32)

    if "nc" not in _compiled:
        _compiled["nc"] = _build_nc()
    nc = _compiled["nc"]

    in_maps = _prepare_in_maps(x, s, style_weight, style_bias, weight, bias)
    res = run_bass_kernel_spmd(nc, in_maps, core_ids=list(range(N_CORES)))

    y = np.empty((N, COUT, DO, DO, DO), np.float32)
    for core in range(N_CORES):
        n, half = divmod(core, 2)
        ys = res.results[core]["y"].reshape(COUT, 2 * PAIRS, DO, DO)
        if half == 0:
            # planes 0..29 full; planes 30,31 rows 0..31 only
            y[n][:, 0:30] = ys[:, 0:30]
            y[n][:, 30:32, 0:32] = ys[:, 30:32, 0:32]
        else:
            # un-mirror: ysf[p', r'] = global (plane 30+p', row r')
            ysf = ys[:, ::-1, ::-1, :]
            y[n][:, 32:DO] = ysf[:, 2:32]
            y[n][:, 30:32, 32:DO] = ysf[:, 0:2, 32:DO]
    return y



# revision 6
# speedup vs baseline: 1.2800x; 1.2800x over previous
"""Styled 3D conv (StyleGAN-style modulated conv3d) on 8 Trainium2 NeuronCores.

Reference computation:
  st = s @ style_weight.T + style_bias                 (N, Cin)
  w  = weight[None] * st[:, None, :, None*3]           (N, Cout, Cin, 3,3,3)
  w  = w * rsqrt(sum(w^2 over (Cin,kd,kh,kw)) + eps)   per-sample demodulated
  y  = grouped_conv3d(x, w, groups=N, VALID) + bias    (N, Cout, 62,62,62)

Shapes: x (4, 64, 64, 64, 64) f32, y (4, 64, 62, 62, 62) f32.

Strategy (8 cores = 4 samples x 2 depth-halves, Winograd F(2,3) along W):
  - Host: modulate + demodulate the per-sample weights, G-transform them
    along kw (3 taps -> 4 Winograd taps), pack matmul lhsT blocks, cast
    x + weights to bf16.
  - Depth split as before: odd core of each sample gets its shard depth+
    height MIRRORED so one SPMD program computes 31/31 output planes.
  - SBUF x slot q = [128 parts = (ci, plane 2q / 2q+1), 64, 64] bf16.
    DVE transforms each slot along W: T0 = x0-x2, T1 = x1+x2, T2 = x2-x1,
    T3 = x1-x3 per 2-wide tile (31 tiles) -> wino slot [128, 64, 4, 31].
  - Output plane pair: M=128 packing (psum 0:64 -> y[d], 64:128 -> y[d+1]),
    K=128 = (ci, 2 planes), the usual 3-tap kd band over 2 slots (75% PE).
    Per 16-row chunk: 24 matmuls (2 slots x 3 kh x 4 wino taps) of
    nrows*31 columns accumulate into 4 psum tap tiles -> 1.5x fewer PE
    columns than direct conv (9 taps over 62 cols -> 4 taps over 31 tiles).
  - Inverse transform on DVE reads the 4 psum tap tiles:
      y_even = m0+m1+m2+bias,  y_odd = m1-m2-m3+bias
    via 2 tensor_tensor + 2 scalar_tensor_tensor ops per chunk, written
    interleaved into the staging tile; DMA to DRAM per pair.

Measured baseline (direct conv): 483 us; this kernel targets ~330 us.
"""

import numpy as np
import ml_dtypes

import concourse.mybir as mybir
import concourse.tile as tile
from concourse import bacc
from concourse.bass_utils import run_bass_kernel_spmd

EPS = 1e-8
N_CORES = 8
N, CIN, COUT, D = 4, 64, 64, 64
DO = D - 2              # 62 output planes/rows/cols
PLANES_IN = 34          # input planes per core
PAIRS = 16              # output plane pairs per core (32 planes)
SLOTS = PLANES_IN // 2  # 17
TILES = 31              # W tiles per row (2 outputs each)
TAPS = 4                # Winograd F(2,3) taps
ROWS_PER_CHUNK = 16
CHUNKS = 4              # 16+16+16+14 = 62 rows
BF16 = mybir.dt.bfloat16
F32 = mybir.dt.float32
ADD = mybir.AluOpType.add
SUB = mybir.AluOpType.subtract
MULT = mybir.AluOpType.mult

_compiled = {}


def _build_nc():
    nc = bacc.Bacc("TRN2", target_bir_lowering=False, debug=False,
                   num_devices=N_CORES)
    xs = nc.dram_tensor("xs", [CIN, PLANES_IN, D, D], BF16,
                        kind="ExternalInput").ap()
    wts = nc.dram_tensor("wts", [128, 24 * 128], BF16,
                         kind="ExternalInput").ap()
    b128 = nc.dram_tensor("b128", [128, 1], F32, kind="ExternalInput").ap()
    y = nc.dram_tensor("y", [COUT, 2 * PAIRS, DO, DO], F32,
                       kind="ExternalOutput").ap()

    with tile.TileContext(nc) as tc:
        with (
            tc.tile_pool(name="wp", bufs=1) as wpool,
            tc.tile_pool(name="xp", bufs=4) as xpool,
            tc.tile_pool(name="wn", bufs=4) as wnpool,
            tc.tile_pool(name="ps", bufs=8, space="PSUM") as pspool,
            tc.tile_pool(name="tm", bufs=6) as tmpool,
            tc.tile_pool(name="st", bufs=2) as stpool,
        ):
            w_sb = wpool.tile([128, 24 * 128], BF16)
            for i in range(4):
                o = i * 768
                nc.sync.dma_start(w_sb[:, o:o + 768], wts[:, o:o + 768])
            bias_sb = wpool.tile([128, 1], F32)
            nc.sync.dma_start(bias_sb[:, :], b128[:, :])

            raw = {}

            def load_raw(q):
                t = xpool.tile([128, D, D], BF16, tag="raw", name=f"raw_{q}")
                nc.sync.dma_start(t[0:64], xs[:, 2 * q])
                nc.sync.dma_start(t[64:128], xs[:, 2 * q + 1])
                raw[q] = t

            wino = {}

            def transform(q, taps=range(TAPS)):
                """W-dir F(2,3) input transform of slot q on DVE."""
                t = raw[q]
                if q not in wino:
                    wino[q] = wnpool.tile([128, D, TAPS, TILES], BF16,
                                          tag="wino", name=f"wino_{q}")
                w = wino[q]
                x0 = t[:, :, 0:62:2]
                x1 = t[:, :, 1:63:2]
                x2 = t[:, :, 2:64:2]
                x3 = t[:, :, 3:64:2]
                for i in taps:
                    a, b, op = ((x0, x2, SUB), (x1, x2, ADD),
                                (x2, x1, SUB), (x1, x3, SUB))[i]
                    nc.vector.tensor_tensor(w[:, :, i, :], a, b, op)

            load_raw(0)
            load_raw(1)
            load_raw(2)

            # PE warmup: keep the HAM clock gate open while the first slots'
            # DMAs + transforms run; sized to finish as wino slot 1 lands.
            warm_w = wpool.tile([128, 128], BF16, name="warm_w")
            warm_x = wpool.tile([128, ROWS_PER_CHUNK, TILES], BF16,
                                name="warm_x")
            nc.vector.memset(warm_w[:, :], 0.0)
            nc.vector.memset(warm_x[:, :, :], 0.0)
            warm_ps = pspool.tile([128, ROWS_PER_CHUNK, TILES], F32,
                                  tag="ps", name="warm_ps")
            for _ in range(130):
                nc.tensor.matmul(warm_ps[:, :, :], warm_w[:, :],
                                 warm_x[:, :, :], start=True, stop=True)

            transform(0)
            transform(1)

            for j in range(PAIRS):
                last = j == PAIRS - 1
                nchunks = 2 if last else CHUNKS
                staging = stpool.tile([128, DO, DO], F32, tag="stage")
                # prefetch: next raw slot + transform one pair ahead,
                # interleaved between chunk combines to keep DVE smooth
                pre = []
                if j + 3 <= SLOTS - 1:
                    load_raw(j + 3)
                if j + 2 <= SLOTS - 1:
                    pre = [(j + 2, i) for i in range(TAPS)]

                for c in range(nchunks):
                    r0 = c * ROWS_PER_CHUNK
                    nrows = min(ROWS_PER_CHUNK, DO - r0)
                    m = [pspool.tile([128, ROWS_PER_CHUNK, TILES], F32,
                                     tag="ps", name=f"ps_{j}_{c}_{i}")
                         for i in range(TAPS)]
                    for var, sl in ((0, j), (1, j + 1)):
                        for kh in range(3):
                            for i in range(TAPS):
                                blk = (var * 3 + kh) * TAPS + i
                                nc.tensor.matmul(
                                    m[i][:, :nrows, :],
                                    w_sb[:, blk * 128:(blk + 1) * 128],
                                    wino[sl][:, r0 + kh:r0 + kh + nrows, i, :],
                                    start=(var == 0 and kh == 0),
                                    stop=(var == 1 and kh == 2))
                    # interleave one prefetch-transform tap per chunk
                    if pre:
                        q, i = pre.pop(0)
                        transform(q, taps=[i])
                        if c == nchunks - 1:
                            while pre:
                                q, i = pre.pop(0)
                                transform(q, taps=[i])
                    # inverse transform + bias (DVE reads at most ONE psum
                    # operand per op — single PSUM read port):
                    #   even cols = m0+m1+m2+b, odd cols = m1-m2-m3+b
                    c1 = tmpool.tile([128, ROWS_PER_CHUNK, TILES], F32,
                                     tag="tmp")
                    t01 = tmpool.tile([128, ROWS_PER_CHUNK, TILES], F32,
                                      tag="tmp")
                    ta = tmpool.tile([128, ROWS_PER_CHUNK, TILES], F32,
                                     tag="tmp")
                    me = [mi[:, :nrows, :] for mi in m]
                    nc.scalar.activation(c1[:, :nrows, :], me[1],
                                         mybir.ActivationFunctionType.Identity,
                                         bias=bias_sb)
                    nc.vector.tensor_tensor(t01[:, :nrows, :], me[0],
                                            c1[:, :nrows, :], ADD)
                    nc.vector.tensor_tensor(
                        staging[:, r0:r0 + nrows, 0:62:2],
                        me[2], t01[:, :nrows, :], ADD)
                    nc.vector.tensor_tensor(ta[:, :nrows, :],
                                            c1[:, :nrows, :], me[2], SUB)
                    nc.vector.scalar_tensor_tensor(
                        staging[:, r0:r0 + nrows, 1:62:2],
                        me[3], -1.0, ta[:, :nrows, :], MULT, ADD)
                    if last:
                        nc.sync.dma_start(y[:, 2 * j, r0:r0 + nrows],
                                          staging[0:64, r0:r0 + nrows])
                        nc.sync.dma_start(y[:, 2 * j + 1, r0:r0 + nrows],
                                          staging[64:128, r0:r0 + nrows])
                if not last:
                    nc.sync.dma_start(y[:, 2 * j], staging[0:64])
                    nc.sync.dma_start(y[:, 2 * j + 1], staging[64:128])
    nc.compile()
    return nc


def _modulated_weights(s_n, style_weight, style_bias, weight):
    st = s_n.astype(np.float32) @ style_weight.T.astype(np.float32) + style_bias
    w = weight * st[None, :, None, None, None]
    demod = 1.0 / np.sqrt(np.sum(w * w, axis=(1, 2, 3, 4)) + EPS)
    return w * demod[:, None, None, None, None]


# F(2,3) weight transform along kw: G @ [w0,w1,w2]
_G = np.array([[1, 0, 0], [0.5, 0.5, 0.5], [0.5, -0.5, 0.5], [0, 0, 1]],
              np.float32)


def _build_lhsT(wmod):
    """(24, 128, 128) fp32 blocks, index (var*3+kh)*4+tap.

    lhsT[k=(half,ci), m=(colhalf,co)]; var 0 = slot j (planes d,d+1),
    var 1 = slot j+1 (planes d+2,d+3); kd band structure as in the
    direct kernel, per (kh, wino tap).
    """
    wwino = np.einsum("ocdhw,tw->ocdht", wmod, _G)  # (co,ci,kd,kh,tap)
    out = np.zeros((3, TAPS, 2, 128, 128), np.float32)
    for kh in range(3):
        for t in range(TAPS):
            wt = wwino[:, :, :, kh, t]         # (co, ci, kd)
            A = out[kh, t, 0]
            B = out[kh, t, 1]
            A[0:64, 0:64] = wt[:, :, 0].T      # lower -> y[d],   kd0
            A[64:128, 0:64] = wt[:, :, 1].T    # upper -> y[d],   kd1
            A[64:128, 64:128] = wt[:, :, 0].T  # upper -> y[d+1], kd0
            B[0:64, 0:64] = wt[:, :, 2].T      # lower -> y[d],   kd2
            B[0:64, 64:128] = wt[:, :, 1].T    # lower -> y[d+1], kd1
            B[64:128, 64:128] = wt[:, :, 2].T  # upper -> y[d+1], kd2
    # reorder to blk = (var*3 + kh)*4 + tap
    out = out.transpose(2, 0, 1, 3, 4).reshape(24, 128, 128)
    return out


def _prepare_in_maps(x, s, style_weight, style_bias, weight, bias):
    bias128 = np.concatenate([bias.reshape(COUT), bias.reshape(COUT)])
    bias128 = np.ascontiguousarray(bias128.reshape(128, 1), np.float32)

    x_bf = x.astype(ml_dtypes.bfloat16)
    in_maps = []
    for core in range(N_CORES):
        n, half = divmod(core, 2)
        wmod = _modulated_weights(s[n], style_weight, style_bias, weight)
        if half == 0:
            xs = x_bf[n][:, 0:PLANES_IN]
        else:
            # mirrored shard: flip depth + height; kd/kh taps flip too,
            # so the same program computes the flipped top half
            xs = x_bf[n][:, D - PLANES_IN:D][:, ::-1, ::-1, :]
            wmod = wmod[:, :, ::-1, ::-1, :]
        lhsT = _build_lhsT(np.ascontiguousarray(wmod))  # (24, 128, 128)
        wts = np.ascontiguousarray(
            lhsT.transpose(1, 0, 2).reshape(128, 24 * 128)
        ).astype(ml_dtypes.bfloat16)
        in_maps.append({"xs": np.ascontiguousarray(xs), "wts": wts,
                        "b128": bias128})
    return in_maps


def kernel(x, s, style_weight, style_bias, weight, bias):
    x = np.asarray(x)
    s = np.asarray(s)
    style_weight = np.asarray(style_weight, np.float32)
    style_bias = np.asarray(style_bias, np.float32)
    weight = np.asarray(weight, np.float32)
    bias = np.asarray(bias, np.float32)

    if "nc" not in _compiled:
        _compiled["nc"] = _build_nc()
    nc = _compiled["nc"]

    in_maps = _prepare_in_maps(x, s, style_weight, style_bias, weight, bias)
    res = run_bass_kernel_spmd(nc, in_maps, core_ids=list(range(N_CORES)))

    y = np.empty((N, COUT, DO, DO, DO), np.float32)
    for core in range(N_CORES):
        n, half = divmod(core, 2)
        ys = res.results[core]["y"].reshape(COUT, 2 * PAIRS, DO, DO)
        if half == 0:
            # planes 0..29 full; planes 30,31 rows 0..31 only
            y[n][:, 0:30] = ys[:, 0:30]
            y[n][:, 30:32, 0:32] = ys[:, 30:32, 0:32]
        else:
            # un-mirror: ysf[p', r'] = global (plane 30+p', row r')
            ysf = ys[:, ::-1, ::-1, :]
            y[n][:, 32:DO] = ysf[:, 2:32]
            y[n][:, 30:32, 32:DO] = ysf[:, 0:2, 32:DO]
    return y
